# revision 17
# baseline (speedup 1.0000x reference)
"""Trainium2 Bass kernel for the LocalConnectivity diamond-ring stencil.

out[b, x, y] = sum_{1<=|dx|+|dy|<=5} w[|dx|+|dy|-1] * in[b, (x+dx)%512, (y+dy)%512]

Strategy
--------
Data-parallel over batch: 64 samples -> 8 cores x 8 samples. Per sample the
512x512 grid is processed in 5 row-tiles (~103 output rows each). The whole
60-tap stencil runs on the TensorEngine as 11 PSUM-accumulating matmuls, one
per horizontal shift dy in [-5, 5]:

  psum[p, f] += sum_c  WB_dy[c, p] * X[c, f + dy_idx]

where X is the input tile with 5 halo rows on each side (contraction dim =
nrows+10 partitions) and 5 circular halo columns on each side (horizontal
shifts become free-dim AP offsets), and WB_dy is the banded Toeplitz matrix
holding the vertical taps of kernel column dy: WB_dy[c, p] = K(c-p-5, dy).

v2: all-bf16 matmuls. f32r weights cannot use FWL or the background weight
buffer, which exposed ~157ns of LDWEIGHTS serialization per matmul (~68us
of the 200us baseline). bf16 weights padded to the full 128 columns satisfy
the FWL trigger (NumWeights==128, non-fp32) and LDWEIGHTS prefetches into
the background weight buffer during the previous matmul. Mixed bf16 x f32r
is rejected by the NEFF compiler, so the moving data is converted f32->bf16
on ScalarE (one extra 113x522 copy per tile, ~0.5us, off the critical
engine). PSUM rows nrows..127 accumulate garbage from the zero-padded band
columns and are never evicted. A dozen warmup matmuls on the weight tile
ramp the PE p-state while the first input tiles are still in flight on DMA.

Bulk HBM traffic is issued from GpSimd (software DGE - the only DGE that
fans transfers out across all 16 SDMA engines; the sync/scalar HW-DGE queues
each serialize on one SDMA engine at ~15-18 GB/s). Transfers stay per-tile
so consecutive DMAs round-robin onto different SDMA queues. Circular column
halos are filled by on-chip ScalarE copies; PSUM eviction runs on VectorE.
"""

import numpy as np
import ml_dtypes

import concourse.bass as bass
import concourse.bacc as bacc
import concourse.mybir as mybir
from concourse import tile
from concourse.bass_utils import run_bass_kernel_spmd

B, H, W = 64, 512, 512
NCORES = 8
BPC = B // NCORES  # samples per core
MAXD = 5
HALO = MAXD
DYS = 2 * MAXD + 1  # 11 horizontal shifts
TR = 103  # rows per tile (last tile: 100)
ROW_TILES = [(0, 103), (103, 103), (206, 103), (309, 103), (412, 100)]
XW = W + 2 * HALO  # 522
WARMUP_MM = 12


def _build_band_weights(dw: np.ndarray) -> np.ndarray:
    """[128, 11*128] bf16: WB[c, j*128 + p] = K(c-p-5, j-5)."""
    wb = np.zeros((128, DYS, 128), dtype=np.float32)
    p = np.arange(128)
    for j in range(DYS):
        dy = j - MAXD
        for dx in range(-MAXD, MAXD + 1):
            d = abs(dx) + abs(dy)
            if 1 <= d <= MAXD:
                c = p + dx + HALO
                valid = (c >= 0) & (c < 128)
                wb[c[valid], j, p[valid]] = dw[d - 1]
    return np.ascontiguousarray(
        wb.reshape(128, DYS * 128).astype(ml_dtypes.bfloat16)
    )


_CACHED_NC = None


def _build_program():
    f32 = mybir.dt.float32
    bf16 = mybir.dt.bfloat16

    nc = bacc.Bacc(None, target_bir_lowering=False)
    x = nc.dram_tensor("x", [BPC, H, W], bf16, kind="ExternalInput")
    wb = nc.dram_tensor("wb", [128, DYS * 128], bf16, kind="ExternalInput")
    y = nc.dram_tensor("y", [BPC, H, W], f32, kind="ExternalOutput")

    with tile.TileContext(nc) as tc:
        with (
            tc.tile_pool(name="wpool", bufs=1) as wpool,
            tc.tile_pool(name="xmpool", bufs=5) as xmpool,
            tc.tile_pool(name="xepool", bufs=6) as xepool,
            tc.tile_pool(name="opool", bufs=4) as opool,
            tc.tile_pool(name="pspool", bufs=7, space=bass.MemorySpace.PSUM) as pspool,
        ):
            wtile = wpool.tile([128, DYS * 128], bf16)
            nc.gpsimd.dma_start(wtile[:], wb[:])

            # PE p-state warmup while the first input tiles are on DMA.
            wpt = pspool.tile([128, W], f32, tag="warm", bufs=1)
            for _ in range(WARMUP_MM):
                nc.tensor.matmul(
                    wpt[:],
                    wtile[0:128, 0:128],
                    wtile[0:128, 0:W],
                    start=True,
                    stop=True,
                )

            def issue_inputs(b):
                """Input DMAs for sample b. 2-sample lookahead keeps these
                ahead of the output DMAs on the in-order GpSimd DGE, so the
                next samples' transfers overlap this sample's compute."""
                xt0 = xepool.tile([128, XW], bf16, tag="xt0", name="xt0")
                #   rows 507..511 then 0..107
                nc.sync.dma_start(
                    xt0[0:HALO, HALO : HALO + W], x[b, H - HALO : H, :]
                )
                nc.gpsimd.dma_start(
                    xt0[HALO : HALO + 108, HALO : HALO + W], x[b, 0:108, :]
                )
                # interior tiles t=1..3: per-tile DMAs (merging them onto
                # one logical queue was measured 1.8x slower end-to-end)
                xtm = xmpool.tile([128, 3, XW], bf16, name="xtm")
                for tt in range(3):
                    r0 = TR * (tt + 1)
                    nc.gpsimd.dma_start(
                        xtm[0:113, tt, HALO : HALO + W],
                        x[b, r0 - HALO : r0 + 108, :],
                    )
                xt4 = xepool.tile([128, XW], bf16, tag="xt4", name="xt4")
                #   rows 407..511 then 0..4
                nc.gpsimd.dma_start(
                    xt4[0:105, HALO : HALO + W], x[b, 4 * TR - HALO : H, :]
                )
                nc.sync.dma_start(xt4[105:110, HALO : HALO + W], x[b, 0:HALO, :])
                return xt0, xtm, xt4

            pending = [issue_inputs(0), issue_inputs(1)]
            done_otb = {}

            def issue_output(bb, t):
                # outputs ride the two HWDGE rings (their own DMAHW sem
                # lanes), keeping the 8 SWDGE lanes free for input tiles
                r0, nrows = ROW_TILES[t]
                eng = nc.sync if t % 2 == 0 else nc.scalar
                eng.dma_start(
                    y[bb, r0 : r0 + nrows, :], done_otb[bb][0:nrows, t, :]
                )

            for b in range(BPC):
                xt0, xtm, xt4 = pending[b]
                if b + 2 < BPC:
                    pending.append(issue_inputs(b + 2))

                # ---- per tile: halo fill + f32->bf16 convert (ScalarE,
                # issued per-tile so tile t's matmuls depend only on tile t's
                # own DMA), 11 accumulating matmuls, eviction, output DMA ----
                otb = opool.tile([128, 5, W], f32)
                for t, (r0, nrows) in enumerate(ROW_TILES):
                    ctr = nrows + 2 * HALO

                    def s(rs, cs, _t=t, _xt0=xt0, _xt4=xt4, _xtm=xtm):
                        if _t == 0:
                            return _xt0[rs, cs]
                        if _t == 4:
                            return _xt4[rs, cs]
                        return _xtm[rs, _t - 1, cs]

                    nc.scalar.copy(
                        s(slice(0, ctr), slice(0, HALO)),
                        s(slice(0, ctr), slice(W, W + HALO)),
                    )
                    nc.scalar.copy(
                        s(slice(0, ctr), slice(HALO + W, XW)),
                        s(slice(0, ctr), slice(HALO, 2 * HALO)),
                    )
                    pt = pspool.tile([128, W], f32)
                    for j in range(DYS):
                        nc.tensor.matmul(
                            pt[:],
                            wtile[0:ctr, j * 128 : (j + 1) * 128],
                            s(slice(0, ctr), slice(j, j + W)),
                            start=(j == 0),
                            stop=(j == DYS - 1),
                        )
                    nc.vector.tensor_copy(otb[0:nrows, t, :], pt[0:nrows, :])
                    # output DMA for the PREVIOUS sample's same tile: its
                    # eviction is long done, so the descriptor-gen never
                    # blocks the GpSimd DGE on an evict semaphore, and the
                    # transfers stream during compute instead of piling up
                    if b >= 1:
                        issue_output(b - 1, t)
                done_otb[b] = otb

            for t in range(len(ROW_TILES)):
                issue_output(BPC - 1, t)
    nc.compile()
    return nc


def _get_program():
    global _CACHED_NC
    if _CACHED_NC is None:
        _CACHED_NC = _build_program()
    return _CACHED_NC


def _run(grid_spikes, distance_weights, trace=False):
    grid_spikes = np.asarray(grid_spikes)
    distance_weights = np.asarray(distance_weights, dtype=np.float32)
    assert grid_spikes.shape == (B, H, W), grid_spikes.shape
    wb_np = _build_band_weights(distance_weights)
    # host-side f32 -> bf16: halves input HBM traffic and removes the
    # on-chip conversion from the per-tile critical chain
    x16 = grid_spikes.astype(ml_dtypes.bfloat16)

    nc = _get_program()
    in_maps = [
        {
            "x": np.ascontiguousarray(x16[i * BPC : (i + 1) * BPC]),
            "wb": wb_np,
        }
        for i in range(NCORES)
    ]
    res = run_bass_kernel_spmd(nc, in_maps, list(range(NCORES)), trace=trace)
    out = np.concatenate([res.results[i]["y"] for i in range(NCORES)], axis=0)
    return out.astype(np.float32, copy=False), res


def kernel(grid_spikes, distance_weights):
    out, _ = _run(grid_spikes, distance_weights, trace=False)
    return out


def kernel_traced(grid_spikes, distance_weights):
    out, res = _run(grid_spikes, distance_weights, trace=True)
    return out, res


# revision 20
# speedup vs baseline: 1.7348x; 1.7348x over previous
"""Trainium2 Bass kernel for the LocalConnectivity diamond-ring stencil.

out[b, x, y] = sum_{1<=|dx|+|dy|<=5} w[|dx|+|dy|-1] * in[b, (x+dx)%512, (y+dy)%512]

Strategy
--------
Data-parallel over batch: 64 samples -> 8 cores x 8 samples. Per sample the
512x512 grid is processed in 5 row-tiles (~103 output rows each), with 5
circular halo rows / columns so vertical taps live in the contraction dim of
PSUM-accumulating TensorEngine matmuls and horizontal shifts are free-dim AP
offsets. Band matrix for shift dy: WB_dy[c, p] = K(c-p-5, dy).

Numerics / PE usage:
- central shifts |dy|<=2 (5 bands): bf16 weights x bf16 data, full 128
  weight columns -> FWL + background-buffer LDWEIGHTS (hidden); 1 col/cycle.
- outer shifts |dy|>=3 (6 bands, taps all <= w3): fp8e4 DoubleRow, two
  bands per matmul at 0.5 cycles/col. The two moving views come from an
  interleaved fp8 tile [128, 2, 527] (two copies of the tile, col pitch
  527) so a (dy, dy+1) pair is one 3D AP with middle stride 528 (16 | 528,
  the DoubleRow constraint). Bands dy=-3 and dy=+5 ride zero-padded pairs.
- input is converted f32->bf16 on the HOST (halves input HBM traffic; the
  NEFF never sees f32 inputs); fp8 copies are made on-chip (ScalarE makes
  copy 0, VectorE copy 1) after ScalarE fills the bf16 circular halos.

Dataflow (the hard-won part -- all measured on HW):
- ALL bulk DMA goes through the GpSimd software DGE: it fans each transfer
  across 16 SDMA engines. The sync/scalar HWDGE rings serialize at
  ~16 GB/s (moving outputs there measured 315us vs 186us) - only the two
  tiny 5-row edge wraps ride sync.
- Every SWDGE DMA instruction waits on one of 8 DMASW semaphore lanes and
  cannot start descriptor-gen until the DMA 8-back fully completes, so at
  most 8 DMAs are in flight; descriptor-gen is ~12ns/row of Q7 time. Input
  tiles are therefore prefetched 2 samples ahead, per-tile (merging tiles
  into one 3D DMA put ~0.7MB on one queue: 1.8x slower end-to-end), and
  output DMAs are issued one sample LATE, one per tile, so their evict
  semaphores are always already satisfied and never stall the DGE.
- A dozen warmup matmuls on the weight tile ramp the PE p-state while the
  first input tiles are in flight (a PE idle gap costs ~3us of 2x-slow
  matmuls afterwards, so gaps are poison twice over).
"""

import numpy as np
import ml_dtypes

import concourse.bass as bass
import concourse.bacc as bacc
import concourse.mybir as mybir
from concourse import tile
from concourse.bass_utils import run_bass_kernel_spmd

B, H, W = 64, 512, 512
NCORES = 8
BPC = B // NCORES  # samples per core
MAXD = 5
HALO = MAXD
DYS = 2 * MAXD + 1  # 11 horizontal shifts
TR = 103  # rows per tile (last tile: 100)
ROW_TILES = [(0, 103), (103, 103), (206, 103), (309, 103), (412, 100)]
XW = W + 2 * HALO  # 522
S8 = 527  # fp8 interleave col pitch: middle AP stride S8+1=528, 16|528
CENTRAL = [3, 4, 5, 6, 7]  # j index (dy = j-5): |dy| <= 2
# fp8 DoubleRow pairs (a, b) = shifts (ja, ja+1); None = zero weights
# (zero halves still read real in-bounds data: 0 x NaN would poison PSUM)
DR_GROUPS = [(0, 1), (8, 9), (None, 2), (None, 10)]
WARMUP_MM = 12


def _band_col(dy: int, dw: np.ndarray) -> np.ndarray:
    """[128, 128] f32 band for horizontal shift dy: WB[c, p] = K(c-p-5, dy)."""
    wbj = np.zeros((128, 128), dtype=np.float32)
    p = np.arange(128)
    for dx in range(-MAXD, MAXD + 1):
        d = abs(dx) + abs(dy)
        if 1 <= d <= MAXD:
            c = p + dx + HALO
            valid = (c >= 0) & (c < 128)
            wbj[c[valid], p[valid]] = dw[d - 1]
    return wbj


def _build_weights(dw: np.ndarray):
    wb = np.stack([_band_col(j - MAXD, dw) for j in CENTRAL], axis=1)
    wb = np.ascontiguousarray(
        wb.reshape(128, len(CENTRAL) * 128).astype(ml_dtypes.bfloat16)
    )
    w8 = np.zeros((128, len(DR_GROUPS), 2, 128), dtype=np.float32)
    for g, (ja, jb) in enumerate(DR_GROUPS):
        if ja is not None:
            w8[:, g, 0, :] = _band_col(ja - MAXD, dw)
        if jb is not None:
            w8[:, g, 1, :] = _band_col(jb - MAXD, dw)
    w8 = np.ascontiguousarray(w8.astype(ml_dtypes.float8_e4m3))
    return wb, w8


_CACHED_NC = None


def _build_program():
    f32 = mybir.dt.float32
    bf16 = mybir.dt.bfloat16
    fp8 = mybir.dt.float8e4

    nc = bacc.Bacc(None, target_bir_lowering=False)
    x = nc.dram_tensor("x", [BPC, H, W], bf16, kind="ExternalInput")
    wb = nc.dram_tensor(
        "wb", [128, len(CENTRAL) * 128], bf16, kind="ExternalInput"
    )
    w8 = nc.dram_tensor(
        "w8", [128, len(DR_GROUPS), 2, 128], fp8, kind="ExternalInput"
    )
    y = nc.dram_tensor("y", [BPC, H, W], f32, kind="ExternalOutput")

    with tile.TileContext(nc) as tc:
        with (
            tc.tile_pool(name="wpool", bufs=1) as wpool,
            tc.tile_pool(name="xmpool", bufs=5) as xmpool,
            tc.tile_pool(name="x8pool", bufs=10) as x8pool,
            tc.tile_pool(name="xepool", bufs=6) as xepool,
            tc.tile_pool(name="opool", bufs=4) as opool,
            tc.tile_pool(name="pspool", bufs=7, space=bass.MemorySpace.PSUM) as pspool,
        ):
            wtile = wpool.tile([128, len(CENTRAL) * 128], bf16)
            nc.gpsimd.dma_start(wtile[:], wb[:])
            w8tile = wpool.tile([128, len(DR_GROUPS), 2, 128], fp8)
            nc.gpsimd.dma_start(w8tile[:], w8[:])

            # PE p-state warmup while the first input tiles are on DMA.
            wpt = pspool.tile([128, W], f32, tag="warm", bufs=1)
            for _ in range(WARMUP_MM):
                nc.tensor.matmul(
                    wpt[:],
                    wtile[0:128, 0:128],
                    wtile[0:128, 0:W],
                    start=True,
                    stop=True,
                )

            def issue_inputs(b):
                """Input DMAs for sample b (prefetched 2 samples ahead)."""
                xt0 = xepool.tile([128, XW], bf16, tag="xt0", name="xt0")
                #   rows 507..511 then 0..107
                nc.sync.dma_start(
                    xt0[0:HALO, HALO : HALO + W], x[b, H - HALO : H, :]
                )
                nc.gpsimd.dma_start(
                    xt0[HALO : HALO + 108, HALO : HALO + W], x[b, 0:108, :]
                )
                xtm = xmpool.tile([128, 3, XW], bf16, name="xtm")
                for tt in range(3):
                    r0 = TR * (tt + 1)
                    nc.gpsimd.dma_start(
                        xtm[0:113, tt, HALO : HALO + W],
                        x[b, r0 - HALO : r0 + 108, :],
                    )
                xt4 = xepool.tile([128, XW], bf16, tag="xt4", name="xt4")
                #   rows 407..511 then 0..4
                nc.gpsimd.dma_start(
                    xt4[0:105, HALO : HALO + W], x[b, 4 * TR - HALO : H, :]
                )
                nc.sync.dma_start(xt4[105:110, HALO : HALO + W], x[b, 0:HALO, :])
                return xt0, xtm, xt4

            pending = [issue_inputs(0), issue_inputs(1)]
            done_otb = {}

            def issue_output(bb, t):
                r0, nrows = ROW_TILES[t]
                nc.gpsimd.dma_start(
                    y[bb, r0 : r0 + nrows, :], done_otb[bb][0:nrows, t, :]
                )

            for b in range(BPC):
                xt0, xtm, xt4 = pending[b]
                if b + 2 < BPC:
                    pending.append(issue_inputs(b + 2))

                otb = opool.tile([128, 5, W], f32)
                for t, (r0, nrows) in enumerate(ROW_TILES):
                    ctr = nrows + 2 * HALO

                    def s(rs, cs, _t=t, _xt0=xt0, _xt4=xt4, _xtm=xtm):
                        if _t == 0:
                            return _xt0[rs, cs]
                        if _t == 4:
                            return _xt4[rs, cs]
                        return _xtm[rs, _t - 1, cs]

                    # circular column halos (bf16, ScalarE)
                    nc.scalar.copy(
                        s(slice(0, ctr), slice(0, HALO)),
                        s(slice(0, ctr), slice(W, W + HALO)),
                    )
                    nc.scalar.copy(
                        s(slice(0, ctr), slice(HALO + W, XW)),
                        s(slice(0, ctr), slice(HALO, 2 * HALO)),
                    )
                    # interleaved fp8 copies for the DoubleRow bands:
                    # copy 0 on ScalarE, copy 1 on VectorE (runs while the
                    # central bf16 matmuls of this tile execute)
                    x8 = x8pool.tile([128, 2, S8], fp8)
                    nc.scalar.copy(
                        x8[0:ctr, 0, 0:XW], s(slice(0, ctr), slice(0, XW))
                    )
                    nc.vector.tensor_copy(
                        x8[0:ctr, 1, 0:XW], s(slice(0, ctr), slice(0, XW))
                    )

                    pt = pspool.tile([128, W], f32)
                    # central bands first: bf16, they only need the halo
                    # copies, so the fp8 converts overlap them
                    for jc, j in enumerate(CENTRAL):
                        nc.tensor.matmul(
                            pt[:],
                            wtile[0:ctr, jc * 128 : (jc + 1) * 128],
                            s(slice(0, ctr), slice(j, j + W)),
                            start=(jc == 0),
                            stop=False,
                        )
                    # outer bands: fp8 DoubleRow, two shifts per matmul via
                    # the interleaved tile (middle stride 528 = 16*33)
                    x8b = x8[0:ctr, 0:2, 0:W]
                    for g, (ja, jb) in enumerate(DR_GROUPS):
                        j0 = ja if ja is not None else jb - 1
                        rhs = bass.AP(
                            x8b.tensor,
                            x8b.offset + j0,
                            [[2 * S8, ctr], [S8 + 1, 2], [1, W]],
                        )
                        nc.tensor.matmul(
                            pt[:],
                            w8tile[0:ctr, g, 0:2, 0:128],
                            rhs,
                            start=False,
                            stop=(g == len(DR_GROUPS) - 1),
                            perf_mode=mybir.MatmulPerfMode.DoubleRow,
                        )
                    nc.vector.tensor_copy(otb[0:nrows, t, :], pt[0:nrows, :])
                    # output DMA for the PREVIOUS sample's same tile: its
                    # eviction is long done, so the descriptor-gen never
                    # blocks the GpSimd DGE on an evict semaphore
                    if b >= 1:
                        issue_output(b - 1, t)
                done_otb[b] = otb

            for t in range(len(ROW_TILES)):
                issue_output(BPC - 1, t)
    nc.compile()
    return nc


def _get_program():
    global _CACHED_NC
    if _CACHED_NC is None:
        _CACHED_NC = _build_program()
    return _CACHED_NC


def _run(grid_spikes, distance_weights, trace=False):
    grid_spikes = np.asarray(grid_spikes)
    distance_weights = np.asarray(distance_weights, dtype=np.float32)
    assert grid_spikes.shape == (B, H, W), grid_spikes.shape
    wb_np, w8_np = _build_weights(distance_weights)
    # host-side f32 -> bf16: halves input HBM traffic and removes the
    # on-chip conversion from the per-tile critical chain
    x16 = grid_spikes.astype(ml_dtypes.bfloat16)

    nc = _get_program()
    in_maps = [
        {
            "x": np.ascontiguousarray(x16[i * BPC : (i + 1) * BPC]),
            "wb": wb_np,
            "w8": w8_np,
        }
        for i in range(NCORES)
    ]
    res = run_bass_kernel_spmd(nc, in_maps, list(range(NCORES)), trace=trace)
    out = np.concatenate([res.results[i]["y"] for i in range(NCORES)], axis=0)
    return out.astype(np.float32, copy=False), res


def kernel(grid_spikes, distance_weights):
    out, _ = _run(grid_spikes, distance_weights, trace=False)
    return out


def kernel_traced(grid_spikes, distance_weights):
    out, res = _run(grid_spikes, distance_weights, trace=True)
    return out, res
